# revision 6
# baseline (speedup 1.0000x reference)
"""DualPathMSA Trainium2 kernel (8-core SPMD).

Problem (hardcoded shapes): x [4,1024,768] f32, qkv1_w/qkv2_w [2304,768],
proj_w [768,768], chain_value_logit scalar. Two attention paths per head,
gate-mixed logits with a log of the two-hop composition A1@A2, plus a
chained value path. Output [4,1024,768] f32.

Sharding: one (batch b, head-half hh) pair per core — core c handles
b = c//2 and heads hh*6..hh*6+5 (hh = c%2). All [B,H,T,T] work is
head-independent; the output projection is split by input features and
the two half-contributions per batch are summed on the host.

Device layout notes:
 - All matmuls contract over the SBUF partition dim.  x is fed
   pre-transposed (xT [768,1024]) so QKV projections contract D.
 - Q/K stored feature-major ([feat, T]) so S = Q@K.T contracts dk=64.
 - Softmaxes skip max-subtraction (logits are O(1): exp can't overflow),
   which lets exp feed straight off PSUM with a fused row-sum accumulator.
 - Smix = 0.5*S1 + 0.875*S2 + 0.5*lse(S1,S2) + 0.5*log(C+eps) is folded to
   exp(0.5*(S1 + 1.75*S2 + ln(E1+E2) + ln(C+eps))) with E_i = exp(S_i);
   per-row constants drop out of the final softmax.  Only Exp/Ln/Copy are
   used on the ACT engine (one table set, no reloads).
 - A1, A2-chain and Amix transposes go through the PE (bf16, identity mm).
 - fp32r for the fp32 matmuls (QKV, S, proj), bf16 for probability matmuls.
"""
import math

import numpy as np

import concourse.bass as bass
import concourse.mybir as mybir
import concourse.tile as _tile_mod
from concourse.tile import TileContext
from concourse.bass_utils import run_bass_kernel_spmd
from bass_rust import ScopedClock


def _legalize_waits(nc):
    """The walrus build here accepts at most ONE sync wait per instruction.
    Tile's scheduler sometimes attaches 2+ (and the exit drain attaches one
    per outstanding proc).  Hoist extra waits onto same-engine nops inserted
    immediately before the over-subscribed instruction — semantically
    identical (waits AND together on the engine's queue)."""
    cur = nc.cur_bb.bb.instructions
    for fn in nc.m.functions:
        for blk in fn.blocks:
            insts = blk.instructions
            i = 0
            while i < len(insts):
                ins = insts[i]
                si = getattr(ins, "sync_info", None)
                ow = list(si.on_wait) if si is not None and si.on_wait else []
                if len(ow) > 1:
                    for w in ow[1:]:
                        carrier = nc.engines[ins.engine].nop()
                        ci = cur.pop()
                        assert ci is carrier.ins
                        if ci.sync_info is None:
                            ci.sync_info = mybir.SyncInfo(on_wait=[w], on_update=[])
                        else:
                            ci.sync_info.on_wait = [w]
                        insts.insert(i, ci)
                        i += 1
                    si.on_wait = ow[:1]
                i += 1


def _patched_drain_and_barrier(self, tick_clock, wait_clock):
    drain_inst = self.nc.sync.drain()
    wait_clock.add_sem_waits(
        drain_inst.ins, ScopedClock({None: tick_clock.global_clock})
    )
    self.nc.all_engine_barrier()
    assert self.sems is not None
    popped = self.nc._tile_sem_poison_stack.pop()
    assert popped is self._sem_poison
    self.nc.clear_and_free_semaphores(list(self.sems.allocated().values()))
    self.nc.all_engine_barrier()
    _legalize_waits(self.nc)


_tile_mod.TileContext._drain_and_barrier = _patched_drain_and_barrier

F32 = mybir.dt.float32
F32R = mybir.dt.float32r
BF16 = mybir.dt.bfloat16
BF16_NP = mybir.dt.np(BF16)
AF = mybir.ActivationFunctionType
OP = mybir.AluOpType

B, T, D = 4, 1024, 768
H, DK = 12, 64
HPC = 6              # heads per core
NB = T // 128        # 8 row blocks
KI = D // 128        # 6 contraction chunks over D
FPC = HPC * DK       # 384 features per core
EPS = 1e-6

_CACHED_NC = None


def _build_nc():
    nc = bass.Bass()
    xT = nc.dram_tensor("xT", [D, T], F32R, kind="ExternalInput")
    w1 = nc.dram_tensor("w1", [D, 3 * FPC], F32R, kind="ExternalInput")
    w2 = nc.dram_tensor("w2", [D, 3 * FPC], F32R, kind="ExternalInput")
    pwT = nc.dram_tensor("pwT", [FPC, D], F32R, kind="ExternalInput")
    ident = nc.dram_tensor("ident", [128, 128], BF16, kind="ExternalInput")
    outT = nc.dram_tensor("outT", [D, T], F32, kind="ExternalOutput")

    with TileContext(nc) as tc:
        with tc.tile_pool(name="persist", bufs=1) as pers, \
             tc.tile_pool(name="pp", bufs=2, space="PSUM") as pp, \
             tc.tile_pool(name="tp", bufs=2, space="PSUM") as tp, \
             tc.tile_pool(name="py", bufs=1, space="PSUM") as py:

            idt = pers.tile([128, 128], BF16, tag="idt", name="idt")
            nc.sync.dma_start(out=idt, in_=ident[:, :])
            epsc = pers.tile([128, 1], F32, tag="epsc", name="epsc")
            nc.vector.memset(epsc, EPS)
            pwts = []
            for fc in range(3):
                t = pers.tile([128, D], F32R, tag=f"pwt{fc}", name=f"pwt{fc}")
                nc.sync.dma_start(out=t, in_=pwT[fc * 128:(fc + 1) * 128, :])
                pwts.append(t)

            # persistent QKV storage: q/k feature-major (tile i = local heads
            # 2i,2i+1), v natural layout, yT output accumulator.
            q1t = [pers.tile([128, T], F32R, tag=f"q1t{i}", name=f"q1t{i}") for i in range(3)]
            k1t = [pers.tile([128, T], F32R, tag=f"k1t{i}", name=f"k1t{i}") for i in range(3)]
            q2t = [pers.tile([128, T], F32R, tag=f"q2t{i}", name=f"q2t{i}") for i in range(3)]
            k2t = [pers.tile([128, T], F32R, tag=f"k2t{i}", name=f"k2t{i}") for i in range(3)]
            v1n = [pers.tile([128, FPC], BF16, tag=f"v1n{i}", name=f"v1n{i}") for i in range(NB)]
            v2n = [pers.tile([128, FPC], BF16, tag=f"v2n{i}", name=f"v2n{i}") for i in range(NB)]
            yts = [pers.tile([128, T], F32R, tag=f"yt{i}", name=f"yt{i}") for i in range(3)]

            # ======== phase 1: QKV projections ========
            with tc.tile_pool(name="qkvw", bufs=1) as qkvw:
                xTs = []
                for ki in range(KI):
                    t = qkvw.tile([128, T], F32R, tag=f"xt{ki}", name=f"xt{ki}")
                    nc.sync.dma_start(out=t, in_=xT[ki * 128:(ki + 1) * 128, :])
                    xTs.append(t)
                w1s, w2s = [], []
                for ki in range(KI):
                    t = qkvw.tile([128, 3 * FPC], F32R, tag=f"w1_{ki}", name=f"w1_{ki}")
                    nc.sync.dma_start(out=t, in_=w1[ki * 128:(ki + 1) * 128, :])
                    w1s.append(t)
                    t = qkvw.tile([128, 3 * FPC], F32R, tag=f"w2_{ki}", name=f"w2_{ki}")
                    nc.sync.dma_start(out=t, in_=w2[ki * 128:(ki + 1) * 128, :])
                    w2s.append(t)

                for ws, qt, kt, vn in ((w1s, q1t, k1t, v1n), (w2s, q2t, k2t, v2n)):
                    # feature-major Q (ft 0..2) and K (ft 3..5)
                    for ft in range(6):
                        ps = pp.tile([128, T], F32, tag="ps", name="ps")
                        for half in range(2):
                            for ki in range(KI):
                                nc.tensor.matmul(
                                    ps[:, half * 512:(half + 1) * 512],
                                    ws[ki][:, ft * 128:(ft + 1) * 128],
                                    xTs[ki][:, half * 512:(half + 1) * 512],
                                    start=(ki == 0), stop=(ki == KI - 1))
                        dest = qt[ft] if ft < 3 else kt[ft - 3]
                        nc.scalar.copy(dest, ps)
                    # V natural layout
                    for tb in range(NB):
                        psv = tp.tile([128, FPC], F32, tag="tp", name="tp")
                        for ki in range(KI):
                            nc.tensor.matmul(
                                psv,
                                xTs[ki][:, tb * 128:(tb + 1) * 128],
                                ws[ki][:, 2 * FPC:3 * FPC],
                                start=(ki == 0), stop=(ki == KI - 1))
                        nc.vector.tensor_copy(vn[tb], psv)

            # ======== phase 2: per-head attention ========
            with tc.tile_pool(name="attn", bufs=1) as attn, \
                 tc.tile_pool(name="vpool", bufs=NB) as vpool, \
                 tc.tile_pool(name="tmpf", bufs=4) as tmpf, \
                 tc.tile_pool(name="tmpbp", bufs=2) as tmpbp, \
                 tc.tile_pool(name="smalls", bufs=4) as smalls:
                for h in range(HPC):
                    ti, po = h // 2, (h % 2) * 64
                    q1 = q1t[ti][po:po + 64, :]
                    k1 = k1t[ti][po:po + 64, :]
                    q2 = q2t[ti][po:po + 64, :]
                    k2 = k2t[ti][po:po + 64, :]

                    a1t = attn.tile([128, NB, T], BF16, tag="a1t", name="a1t")    # A1.T  [k,(kb,n)]
                    a2t = attn.tile([128, NB, T], BF16, tag="trB", name="a2t")    # A2.T  [m,(mb,k)]
                    a2b = attn.tile([128, NB, T], BF16, tag="a2b", name="a2b")    # A2    [k,(kb,m)]
                    w_sb = attn.tile([128, NB, DK], BF16, tag="wsb", name="wsb")  # A2@V2 [k,(kb,d)]
                    vts = []

                    # ---- stage A: S1/S2, exps, partial Smix, transposes ----
                    for n in range(NB):
                        ps1 = pp.tile([128, T], F32, tag="ps", name="ps")
                        ps2 = pp.tile([128, T], F32, tag="ps", name="ps")
                        for half in range(2):
                            sl = slice(half * 512, (half + 1) * 512)
                            nc.tensor.matmul(ps1[:, sl], q1[:, n * 128:(n + 1) * 128],
                                             k1[:, sl], start=True, stop=True)
                            nc.tensor.matmul(ps2[:, sl], q2[:, n * 128:(n + 1) * 128],
                                             k2[:, sl], start=True, stop=True)
                        e1 = tmpf.tile([128, T], F32, tag="tmp", name="e1")
                        e2 = tmpf.tile([128, T], F32, tag="tmp", name="e2")
                        s1c = tmpf.tile([128, T], F32, tag="tmp", name="s1c")
                        sm = smalls.tile([128, 8], F32, tag="sm", name="sm")
                        r1, r2 = sm[:, 0:1], sm[:, 1:2]
                        vr1, vr2 = sm[:, 2:3], sm[:, 3:4]
                        nc.scalar.activation(e1, ps1, AF.Exp, accum_out=r1)
                        nc.scalar.copy(s1c, ps1)
                        nc.scalar.activation(e2, ps2, AF.Exp, accum_out=r2)
                        vn_t = vpool.tile([128, T], F32, tag="v", name="v")
                        # v = S1 + 1.75*S2
                        nc.vector.scalar_tensor_tensor(vn_t, ps2, 1.75, s1c,
                                                       op0=OP.mult, op1=OP.add)
                        nc.vector.reciprocal(vr1, r1)
                        nc.vector.reciprocal(vr2, r2)
                        a1bf = tmpbp.tile([128, T], BF16, tag="tmpb", name="a1bf")
                        nc.vector.tensor_scalar_mul(a1bf, e1, vr1)
                        nc.vector.tensor_scalar_mul(a2b[:, n, :], e2, vr2)
                        # v += ln(E1+E2)
                        z = tmpf.tile([128, T], F32, tag="tmp", name="z")
                        nc.gpsimd.tensor_tensor(z, e1, e2, op=OP.add)
                        lnz = tmpf.tile([128, T], F32, tag="tmp", name="lnz")
                        nc.scalar.activation(lnz, z, AF.Ln)
                        nc.vector.tensor_tensor(vn_t, vn_t, lnz, op=OP.add)
                        vts.append(vn_t)
                        # transposes: A1 row-block n and A2 row-block (k-block) n
                        pt1 = tp.tile([128, T], BF16, tag="tp", name="pt1")
                        for kb in range(NB):
                            nc.tensor.transpose(pt1[:, kb * 128:(kb + 1) * 128],
                                                a1bf[:, kb * 128:(kb + 1) * 128], idt)
                        nc.vector.tensor_copy(a1t[:, :, n * 128:(n + 1) * 128],
                                              pt1.rearrange("p (b c) -> p b c", c=128))
                        pt2 = tp.tile([128, T], BF16, tag="tp", name="pt2")
                        for mb in range(NB):
                            nc.tensor.transpose(pt2[:, mb * 128:(mb + 1) * 128],
                                                a2b[:, n, mb * 128:(mb + 1) * 128], idt)
                        nc.vector.tensor_copy(a2t[:, :, n * 128:(n + 1) * 128],
                                              pt2.rearrange("p (b c) -> p b c", c=128))

                    # ---- W = A2 @ V2 (per k-block); a2t free afterwards ----
                    for kb in range(NB):
                        psw = tp.tile([128, DK], F32, tag="tp", name="psw")
                        for mb in range(NB):
                            nc.tensor.matmul(psw, a2t[:, mb, kb * 128:(kb + 1) * 128],
                                             v2n[mb][:, h * DK:(h + 1) * DK],
                                             start=(mb == 0), stop=(mb == NB - 1))
                        nc.vector.tensor_copy(w_sb[:, kb, :], psw)

                    # ---- stage B: C = A1@A2, ln, Emix, transposes ----
                    amixt = attn.tile([128, NB, T], BF16, tag="trB", name="amixt")  # Amix.T
                    for n in range(NB):
                        psc = pp.tile([128, T], F32, tag="ps", name="psc")
                        for half in range(2):
                            sl = slice(half * 512, (half + 1) * 512)
                            for kb in range(NB):
                                nc.tensor.matmul(psc[:, sl],
                                                 a1t[:, kb, n * 128:(n + 1) * 128],
                                                 a2b[:, kb, sl],
                                                 start=(kb == 0), stop=(kb == NB - 1))
                        lnc = tmpf.tile([128, T], F32, tag="tmp", name="lnc")
                        nc.scalar.activation(lnc, psc, AF.Ln, bias=epsc)
                        vn_t = vts[n]
                        nc.vector.tensor_tensor(vn_t, vn_t, lnc, op=OP.add)
                        emix = tmpf.tile([128, T], F32, tag="tmp", name="emix")
                        smb = smalls.tile([128, 8], F32, tag="sm", name="smb")
                        rmix, vrm = smb[:, 0:1], smb[:, 1:2]
                        nc.scalar.activation(emix, vn_t, AF.Exp, scale=0.5,
                                             accum_out=rmix)
                        nc.vector.reciprocal(vrm, rmix)
                        ambf = tmpbp.tile([128, T], BF16, tag="tmpb", name="ambf")
                        nc.vector.tensor_scalar_mul(ambf, emix, vrm)
                        ptm = tp.tile([128, T], BF16, tag="tp", name="ptm")
                        for mb in range(NB):
                            nc.tensor.transpose(ptm[:, mb * 128:(mb + 1) * 128],
                                                ambf[:, mb * 128:(mb + 1) * 128], idt)
                        nc.vector.tensor_copy(amixt[:, :, n * 128:(n + 1) * 128],
                                              ptm.rearrange("p (b c) -> p b c", c=128))

                    # ---- y.T[h] = V1.T@Amix.T + W.T@A1.T  -> yts ----
                    psy = py.tile([64, T], F32, tag="py", name="psy")
                    for half in range(2):
                        sl = slice(half * 512, (half + 1) * 512)
                        for mb in range(NB):
                            nc.tensor.matmul(psy[:, sl], v1n[mb][:, h * DK:(h + 1) * DK],
                                             amixt[:, mb, sl], start=(mb == 0), stop=False)
                        for kb in range(NB):
                            nc.tensor.matmul(psy[:, sl], w_sb[:, kb, :],
                                             a1t[:, kb, sl], start=False, stop=(kb == NB - 1))
                    nc.scalar.copy(yts[ti][po:po + 64, :], psy)

                # ======== phase 3: output projection ========
                for jb in range(6):
                    psp = pp.tile([128, T], F32, tag="ps", name="psp")
                    for half in range(2):
                        sl = slice(half * 512, (half + 1) * 512)
                        for fc in range(3):
                            nc.tensor.matmul(psp[:, sl],
                                             pwts[fc][:, jb * 128:(jb + 1) * 128],
                                             yts[fc][:, sl],
                                             start=(fc == 0), stop=(fc == 2))
                    osb = tmpf.tile([128, T], F32, tag="tmp", name="osb")
                    nc.scalar.copy(osb, psp)
                    nc.sync.dma_start(out=outT[jb * 128:(jb + 1) * 128, :], in_=osb)
    return nc


def _get_nc():
    global _CACHED_NC
    if _CACHED_NC is None:
        _CACHED_NC = _build_nc()
    return _CACHED_NC


def _prep_inputs(x, qkv1_w, qkv2_w, proj_w, chain_value_logit):
    x = np.asarray(x, dtype=np.float32)
    qkv1_w = np.asarray(qkv1_w, dtype=np.float32)
    qkv2_w = np.asarray(qkv2_w, dtype=np.float32)
    proj_w = np.asarray(proj_w, dtype=np.float32)
    logit = float(np.asarray(chain_value_logit))
    sig = 1.0 / (1.0 + math.exp(-logit))
    ident = np.eye(128, dtype=BF16_NP)

    in_maps = []
    for c in range(8):
        b, hh = c // 2, c % 2
        fs = slice(hh * FPC, hh * FPC + FPC)
        xT_c = np.ascontiguousarray(x[b].T)
        w_cs = []
        for qw in (qkv1_w, qkv2_w):
            wc = np.concatenate([qw[0 * D:][fs], qw[1 * D:][fs], qw[2 * D:][fs]], axis=0)
            w_cs.append(np.ascontiguousarray(wc.T))
        w1_c, w2_c = w_cs
        w1_c[:, :FPC] *= 0.125          # S1 scale folded into Q1
        w2_c[:, :FPC] *= 0.125          # S2 scale folded into Q2
        w2_c[:, 2 * FPC:] *= sig        # sigmoid(chain_value_logit) folded into V2
        pwT_c = np.ascontiguousarray(proj_w[:, fs].T)
        in_maps.append({"xT": xT_c, "w1": w1_c, "w2": w2_c, "pwT": pwT_c,
                        "ident": ident})
    return in_maps


def _run(inputs, trace=False):
    nc = _get_nc()
    in_maps = _prep_inputs(**inputs)
    res = run_bass_kernel_spmd(nc, in_maps, list(range(8)), trace=trace)
    out = np.empty((B, T, D), dtype=np.float32)
    for b in range(B):
        out[b] = (res.results[2 * b]["outT"] + res.results[2 * b + 1]["outT"]).T
    return out, res


def kernel(**inputs):
    out, _ = _run(inputs)
    return out
